# revision 43
# baseline (speedup 1.0000x reference)
"""Trainium2 Bass kernel for the non-local attention block (dense_transformer).

Reference computation per batch item b (x: [B=32, C=64, H=32, W=32], N=1024):
    xf    = x[b] reshaped [C, N]
    phi   = w_phi   @ xf                     [C, N]
    theta = (w_theta @ xf)^T                 [N, C]
    g     = (w_g @ xf)^T @ w_mv^T            [N, C]
    att   = theta @ phi                      [N, N]
    att   = att @ w_mk^T                     [N, N]
    att   = softmax(att, axis over rows n)
    out   = att @ g                          [N, C]
    final = w_mask @ out^T + xf              [C, N]

Key algebraic restructure: (theta @ phi) @ w_mk^T == theta @ (phi @ w_mk^T),
which removes the N^3 matmul.  The softmax denominator divide is folded into
the small g factor (64 wide).

Per-core layout (data-parallel, 4 batch items per core, as 2 stacked pairs
occupying the 128 SBUF partitions: batch "b" on partitions 0-63, batch "c"
on 64-127).  The four [64,64] conv weights are replicated into [128,128]
BLOCK-DIAGONAL matrices so one full-array matmul computes both batches at
once (stage-1 PhiT/T/GT and the final mask):
    PhiT_il = x_il^T @ bd(w_phi^T)    [n, c2]   8 MMs/pair (m-chunks)
    T       = bd(w_theta^T)^T @ x_il  [c2, n]   2 MMs/pair
    GT_il   = x_il^T @ bd(w_gv^T)     [n, c2]   8 MMs/pair
    P2      = PhiT_il^T @ w_mk^T      [c2, k]   32 full-array MMs/pair
    S       = P2^T @ T (quadrant 4x)  [k, n]    = att2^T, 8 MMs/chunk
    E       = exp(S)  (ScalarE, fused row-sum accum -> D)
    GTs     = GT_il * (1/D)           (fold softmax divide into g)
    O       = GTs^T @ E               [c2, k]   col-split, accum m-chunks
    final   = bd(w_mask^T)^T @ O + x  (mask matmul IN-PLACE into the psO
                                       banks, so no extra PSUM pool)

Schedule (91us baseline -> ~72us; bottlenecks measured from NTFF traces):
  - exp is the hard bottleneck: 32 x [128,1024] ACTIVATEs on the 1.2GHz
    ScalarE (~1.34us each + 0.34us fused-accum read).  8 of the 32
    chunk-batches compute exp on the DVE instead via the Schraudolph
    int16/bf16 bit trick (plus an explicit DVE row-sum), which balances
    ScalarE vs DVE at ~40us each and shortens the exp phase accordingly.
  - All input DMAs on ONE Sync-ring queue in priority order (wbd, x16,
    wmk quarters, xfall): a single queue serializes transfers so the
    pipeline-gating pieces get the full ~300GB/s instead of fair-sharing
    with 2MB of wmk (parallel rings measured xball landing 4-8us later).
  - ~26 warmup matmuls bridge the HAM clock gate (PE at 1.2GHz until
    ~3.4us of sustained activity) across the DMA wait so stage-1 runs at
    2.4GHz; the exp phase then keeps the PE warm end-to-end.
  - Only P2 quarter 0 runs in the lead-in; quarters 1-3 are deferred into
    the exp phase (low priority + dep edges after the S chain).
  - Each pair's O accumulation rides its OWN exp phase (o_chunk(p, k-1)
    after chunk k's S matmuls), so the tail is just the last O chunk +
    mask + out-DMA instead of a 16us cold-PE epilogue.
  - finish(p) runs at normal priority at the next pair's k==0 so the psO
    banks hand off in time and the out-DMA leaves mid-kernel; the tail
    finish uses ScalarE (idle after the last exp) for the O copies.
  - S chunk issues batch-b's 4 quadrant MMs before batch-c's so exp_b's
    operands complete ~430ns earlier (the exp cadence is set by when the
    S matmuls land).

All matmul operands bf16 (PE full rate); PSUM fp32; softmax sums fp32.
PSUM budget (8 banks): psS 2 slots x [128,1024] = 4; psO (O accum +
in-place mask) 2 slots x [128,512] = 2; psSm (stage-1 psums + P2
quarters) 2 slots x [128,512] = 2.

Post-passes: _eliminate_redundant_waits strips Tile's transitively-implied
same-engine sem waits; _split_matmul_waits hoists remaining multi-wait
instructions onto single-wait NoOps (TRN2 walrus allows one sync-wait per
instruction).

Rel err ~1.0e-2 vs the fp32 reference (bf16 matmul rounding).
"""

import numpy as np
import ml_dtypes

import concourse.bass as bass
import concourse.mybir as mybir
import concourse.tile as tile
from concourse.bass_utils import run_bass_kernel_spmd

BF = mybir.dt.bfloat16
F32 = mybir.dt.float32
EXP = mybir.ActivationFunctionType.Exp

B, C, HH, WW = 32, 64, 32, 32
N = HH * WW          # 1024
NCORES = 8
BPC = B // NCORES    # 4 batch items per core
NPAIRS = BPC // 2    # 2 stacked pairs per core
NK = N // 128        # 8 chunks of 128 along the N dimension
NH = 512             # matmul free-dim half (one PSUM bank)


def _build_body(nc, tc, consts, acts, bigacts, psO_pool, psS, psSm,
                xall32, xall16, wbdT, wmkhT, out_e):
    from concourse.bass import _add_dep_helper
    lo = slice(0, 64)
    hi = slice(64, 128)

    # ---- PE warmup: dummy matmuls on a zeroed tile keep the PE busy while
    # the input DMAs run (HAM warm + no MID-window re-throttle before
    # stage-1), sized to end about when xball lands (~11us).
    warm_in = consts.tile([128, 256], BF, tag="warm_in")
    nc.vector.memset(warm_in[:], 0.0)
    warm_ps = psS.tile([128, N], F32, tag="psS", name="warm_ps")
    for i in range(38):
        nc.tensor.matmul(warm_ps[:, 0:128], lhsT=warm_in[:, 0:128],
                         rhs=warm_in[:, 128:256])

    # ---- input DMAs: ALL on the Sync ring in strict priority order.  A
    # single queue serializes the transfers, so the critical pieces (wbd,
    # xball, wmk q0) get the full ~300GB/s aggregate instead of competing
    # with later pieces; each ~0.7us issue overlaps the previous transfer.
    wbd = consts.tile([128, 4 * 128], BF, tag="wbd")
    xball = consts.tile([128, NPAIRS, N], BF, tag="xball")
    x16r = xall16.rearrange("(p q) n -> q p n", p=NPAIRS)
    wth = wbd[:, 0:128]
    wph = wbd[:, 128:256]
    wgv = wbd[:, 256:384]
    wma = wbd[:, 384:512]
    # w_mk^T in k-quarter-major DRAM layout: one piece per quarter so P2
    # quarter j starts as soon as piece j lands (needed at chunk 2j).
    wmk_q = [consts.tile([128, NK, 256], BF, tag=f"wmkq{j}", name=f"wmkq{j}")
             for j in range(4)]
    xfall = consts.tile([128, NPAIRS, N], F32, tag="xfall")

    def wmk_piece(j):
        nc.sync.dma_start(
            wmk_q[j][:], wmkhT[j * N:(j + 1) * N, :].rearrange(
                "(mc q) k -> q mc k", mc=NK))

    # x pair 0 before wmk q0 before x pair 1: stage-1 needs only pair 0's
    # x, so the P2-gating wmk piece overtakes the second x half.
    nc.sync.dma_start(wbd[:], wbdT[:])
    nc.sync.dma_start(xball[:, 0, :], x16r[:, 0, :])
    wmk_piece(0)
    nc.sync.dma_start(xball[:, 1, :], x16r[:, 1, :])
    wmk_piece(1)
    wmk_piece(2)
    wmk_piece(3)
    nc.sync.dma_start(xfall[:], xall32.rearrange("(p q) n -> q p n", p=NPAIRS))

    st = [dict() for _ in range(NPAIRS)]

    def stage1(p, lead=False, after=None):
        """PhiT_il + T for pair p (block-diagonal weights, full-array MMs).
        In the lead-in the psum copies alternate ScalarE/VectorE so they
        parallelize (ScalarE is idle before the first exp)."""
        xb = xball[:, p, :]
        s = st[p]

        def sc_copy(out, in_):
            nc.scalar.copy(out, in_)

        def ve_copy(out, in_):
            nc.vector.tensor_copy(out=out, in_=in_)

        copiers = [sc_copy, ve_copy] if lead else [ve_copy, ve_copy]
        PhiT = acts.tile([128, NK, 128], BF, tag="PhiT", name="PhiT")
        for g in range(2):
            psPh = psSm.tile([128, 4, 128], F32, tag="psSm", name="psPh")
            for mq in range(4):
                m = g * 4 + mq
                mm = nc.tensor.matmul(psPh[:, mq, :],
                                      lhsT=xb[:, m * 128:(m + 1) * 128],
                                      rhs=wph[:])
                if after is not None:
                    _add_dep_helper(mm.ins, after.ins, reason="stage1 after S")
                    after = None
            copiers[g](PhiT[:, g * 4:(g + 1) * 4, :], psPh[:])
        T_sb = acts.tile([128, N], BF, tag="T_sb", name="T_sb")
        for h in range(2):
            hh = slice(h * NH, (h + 1) * NH)
            psT = psSm.tile([128, NH], F32, tag="psSm", name="psT")
            nc.tensor.matmul(psT[:], lhsT=wth[:], rhs=xb[:, hh])
            copiers[h](T_sb[:, hh], psT[:])
        s.update(PhiT=PhiT, T_sb=T_sb)
        s["P2"] = acts.tile([128, N], BF, tag="P2", name="P2")

    def gtstage(p, after=None):
        """GT_il for pair p — off the first-exp critical path."""
        xb = xball[:, p, :]
        s = st[p]
        GT = acts.tile([128, NK, 128], BF, tag="GT", name="GT")
        for g in range(2):
            psG = psSm.tile([128, 4, 128], F32, tag="psSm", name="psG")
            for mq in range(4):
                m = g * 4 + mq
                mm = nc.tensor.matmul(psG[:, mq, :],
                                      lhsT=xb[:, m * 128:(m + 1) * 128],
                                      rhs=wgv[:])
                if after is not None:
                    _add_dep_helper(mm.ins, after.ins, reason="GT after S")
                    after = None
            nc.vector.tensor_copy(out=GT[:, g * 4:(g + 1) * 4, :], in_=psG[:])
        s["GT"] = GT

    def p2_quarter(p, j, after=None, lead=False):
        """P2 column-quarter j (256 k's) for pair p — one full-array MM per
        m-chunk (both batches via the interleaved PhiT layout)."""
        s = st[p]
        psP2 = psSm.tile([128, 256], F32, tag="psSm", name="psP2")
        for m in range(NK):
            mm = nc.tensor.matmul(psP2[:], lhsT=s["PhiT"][:, m, :],
                                  rhs=wmk_q[j][:, m, :],
                                  start=(m == 0), stop=(m == NK - 1))
            if after is not None:
                _add_dep_helper(mm.ins, after.ins, reason="P2 after S chain")
                after = None
        if lead:
            nc.scalar.copy(s["P2"][:, j * 256:(j + 1) * 256], psP2[:])
        else:
            nc.vector.tensor_copy(out=s["P2"][:, j * 256:(j + 1) * 256],
                                  in_=psP2[:])

    def alloc_e(p):
        s = st[p]
        s["E_b"] = bigacts.tile([128, NK, N], BF, tag="E_b", name="E_b")
        s["E_c"] = bigacts.tile([128, NK, N], BF, tag="E_c", name="E_c")
        s["D"] = acts.tile([128, NK, 2], F32, tag="D", name="D")
        s["R"] = acts.tile([128, NK, 2], F32, tag="R", name="R")
        s["GTs"] = acts.tile([128, NK, 128], BF, tag="GTs", name="GTs")

    def alloc_o(p):
        s = st[p]
        s["psO"] = [psO_pool.tile([128, NH], F32, tag="psO", name=f"psO{h}")
                    for h in range(2)]

    # Schraudolph fast-exp constants for a bf16 bit pattern: computing
    # round(SCH_A*x + SCH_B) as int16 and reinterpreting the bits as bf16
    # yields exp(x) to ~2-3% relative error.  The systematic error largely
    # cancels in the softmax ratio E/sum(E) (verified vs the fp32
    # reference: rel err 0.0101 -> 0.0104 with 8/32 chunk-batches
    # offloaded).  This moves exp work off the bottleneck ScalarE onto the
    # otherwise-slack DVE.  Inputs here span [-75, +75], safely inside the
    # valid [-87, +88] window of the trick.
    SCH_A = 128.0 / float(np.log(2.0))
    SCH_B = 127.0 * 128.0 - 0.0450466 * 128.0

    def s_exp_chunk(p, k):
        """S matmuls + exp (fused row-sum) for k-chunk of pair p.

        Four 64x64 PE quadrants via tile-position packing; batch b's four
        matmuls issue first so exp_b's operands land ~430ns earlier.
        Batch c of even chunks computes exp on the DVE (Schraudolph) with
        an explicit DVE row-sum reduce; the rest use ScalarE exp with the
        fused accumulator row-sum.
        """
        s = st[p]
        kk = slice(k * 128, (k + 1) * 128)
        psS_b = psS.tile([128, N], F32, tag="psS", name="psS_b")
        psS_c = psS.tile([128, N], F32, tag="psS", name="psS_c")
        # One [64,128]-stationary matmul per (batch, half) writes all 128
        # k-partitions at once: same array cycles as the old 2-quadrant
        # col-split (row util is 50% either way — rank-64 contraction) but
        # half the LDWEIGHTS/MATMUL instructions, so exp_b's operands land
        # sooner after the psS slot frees.
        for h in range(2):
            hh = slice(h * NH, (h + 1) * NH)
            nc.tensor.matmul(psS_b[:, hh], lhsT=s["P2"][lo, kk],
                             rhs=s["T_sb"][lo, hh])
        last = None
        for h in range(2):
            hh = slice(h * NH, (h + 1) * NH)
            last = nc.tensor.matmul(psS_c[:, hh], lhsT=s["P2"][hi, kk],
                                    rhs=s["T_sb"][hi, hh])
        nc.scalar.activation(s["E_b"][:, k, :], psS_b[:], EXP,
                             accum_out=s["D"][:, k, 0:1])
        if k % 2 == 0:
            ec_i16 = s["E_c"][:, k, :].bitcast(mybir.dt.int16)
            nc.vector.tensor_scalar(out=ec_i16, in0=psS_c[:],
                                    scalar1=SCH_A, scalar2=SCH_B,
                                    op0=mybir.AluOpType.mult,
                                    op1=mybir.AluOpType.add)
            nc.vector.tensor_reduce(out=s["D"][:, k, 1:2],
                                    in_=s["E_c"][:, k, :],
                                    axis=mybir.AxisListType.X,
                                    op=mybir.AluOpType.add)
        else:
            nc.scalar.activation(s["E_c"][:, k, :], psS_c[:], EXP,
                                 accum_out=s["D"][:, k, 1:2])
        return last

    def gts_chunk(p, k):
        s = st[p]
        nc.vector.reciprocal(s["R"][:, k, :], s["D"][:, k, :])
        nc.vector.tensor_scalar_mul(s["GTs"][:, k, 0:64], s["GT"][:, k, 0:64],
                                    s["R"][:, k, 0:1])
        nc.vector.tensor_scalar_mul(s["GTs"][:, k, 64:128],
                                    s["GT"][:, k, 64:128], s["R"][:, k, 1:2])

    def o_chunk(p, m, after=None):
        """O accumulation m-chunk for pair p (col-split, both batches).
        `after`: S matmul the first O matmul must follow in the PE stream
        (keeps cold O matmuls from head-of-line-blocking the exp chain)."""
        s = st[p]
        for h in range(2):
            hh = slice(h * NH, (h + 1) * NH)
            mm = nc.tensor.matmul(s["psO"][h][lo, :],
                                  lhsT=s["GTs"][:, m, 0:64],
                                  rhs=s["E_b"][:, m, hh],
                                  start=(m == 0), stop=(m == NK - 1))
            if after is not None:
                _add_dep_helper(mm.ins, after.ins, reason="O chunk after S")
                after = None
            nc.tensor.matmul(s["psO"][h][hi, :], lhsT=s["GTs"][:, m, 64:128],
                             rhs=s["E_c"][:, m, hh],
                             start=(m == 0), stop=(m == NK - 1))

    def finish(p, after=None, tail=False):
        """O copyback, in-place mask matmul (reuses psO banks), residual
        add, out DMA for pair p.  Half-pipelined: h0's chain runs while
        h1's O matmuls finish.  In the tail, ScalarE (idle after the last
        exp) takes the O copies off the DVE."""
        s = st[p]
        O_sb = acts.tile([128, N], BF, tag="O_sb", name="O_sb")
        out_sb = acts.tile([128, N], F32, tag="out_sb", name="out_sb")
        for h in range(2):
            hh = slice(h * NH, (h + 1) * NH)
            # O copies on ScalarE: finish is emitted inline (normal prio),
            # so these land mid-stream on the slack ACT FIFO, off the DVE.
            nc.scalar.copy(O_sb[:, hh], s["psO"][h][:])
            mm = nc.tensor.matmul(s["psO"][h][:], lhsT=wma[:], rhs=O_sb[:, hh],
                                  start=True, stop=True)
            if after is not None:
                _add_dep_helper(mm.ins, after.ins, reason="mask after S")
                after = None
            nc.vector.tensor_tensor(out_sb[:, hh], s["psO"][h][:],
                                    xfall[:, p, hh], mybir.AluOpType.add)
            # Half-DMA right after each half's residual add: h0's 256KB is
            # in flight while h1's mask/add still compute, shaving the
            # serial tail after the last exp.
            nc.gpsimd.dma_start(out_e[p * 128:(p + 1) * 128, hh],
                                out_sb[:, hh])

    def low():
        return tc.high_priority(offset=-100000)

    # ---- software pipeline ----
    # Deferred matmul groups run at low priority (the Tile scheduler slots
    # them into PE idle gaps) with dep edges (after=) keeping them behind
    # the current chunk's S chain in the PE FIFO.
    stage1(0, lead=True)
    alloc_e(0)
    p2_quarter(0, 0, lead=True)
    # O chunks run TWO windows behind their exp chunk: their inputs (E and
    # the scaled GTs) are then always stale, so a lagging DVE can never
    # make an O group head-of-line-block the next S chain in the PE FIFO.
    for p in range(NPAIRS):
        nxt = p + 1
        alloc_o(p)
        for k in range(NK):
            s_mm = s_exp_chunk(p, k)
            if k == 0:
                if p == 0:
                    with low():
                        gtstage(0, after=s_mm)
                else:
                    with low():
                        o_chunk(p - 1, NK - 2, after=s_mm)
            if k == 1 and p > 0:
                with low():
                    o_chunk(p - 1, NK - 1, after=s_mm)
                finish(p - 1, after=s_mm)
            gts_chunk(p, k)
            if k >= 2:
                with low():
                    o_chunk(p, k - 2, after=s_mm)
            with low():
                if k == 0:
                    p2_quarter(p, 1, after=s_mm)
                if k == 1:
                    p2_quarter(p, 2, after=s_mm)
                if k == 3:
                    p2_quarter(p, 3, after=s_mm)
            if nxt < NPAIRS:
                if k == 2:
                    with low():
                        stage1(nxt, after=s_mm)
                if k == 4:
                    alloc_e(nxt)
                    with low():
                        gtstage(nxt, after=s_mm)
                if k == 5:
                    # Normal priority: emitted inline between chunk 5 and
                    # 6, so the P2->cast->S(nxt,0) chain completes well
                    # before the pair transition instead of running at the
                    # low-priority stream tail right when it's needed.
                    p2_quarter(nxt, 0, after=s_mm)
    o_chunk(NPAIRS - 1, NK - 2)
    o_chunk(NPAIRS - 1, NK - 1)
    finish(NPAIRS - 1, tail=True)


def _eliminate_redundant_waits(nc):
    """Transitive redundant-wait elimination over the final BIR stream.

    Tile's sem assignment is per-proc minimal but NOT transitively minimal:
    e.g. a matmul reusing a PSUM slot gets both (ACT >= k) [reader done] and
    (PE >= p) [previous writer done] waits, although observing ACT >= k
    already implies PE >= p (the reader waited on the writer).  The extra
    same-engine waits serialize the PE pipeline (no back-to-back streaming,
    no quadrant concurrency).

    Soundness relies on per-queue in-order completion (PE pc-monotone,
    ACT/DVE strict FIFO):  observing sem s >= v implies the v-th
    incrementing instruction and its whole same-queue prefix completed,
    hence all THEIR increments fired and all their waits were satisfied.
    """
    blocks = list(nc.m.functions[0].blocks)
    seq = []
    for blk in blocks:
        for ins in blk.instructions:
            seq.append(ins)

    def queue_key(ins):
        si = getattr(ins, "sync_info", None)
        nm = type(ins).__name__
        if nm in ("InstDMACopy", "InstTensorLoad", "InstTensorSave"):
            if si and si.on_update:
                return "Q" + si.on_update[0].ant_name
        return "E" + str(ins.engine)

    sem_count = {}
    incpoints = {}
    qpos = {}
    qidx = {}
    for ins in seq:
        qk = queue_key(ins)
        i = qpos.get(qk, 0)
        qidx[id(ins)] = (qk, i)
        qpos[qk] = i + 1
        si = getattr(ins, "sync_info", None)
        if si and si.on_update:
            for u in si.on_update:
                s = u.ant_name
                v = sem_count.get(s, 0) + (u.update_value or 1)
                sem_count[s] = v
                incpoints.setdefault(s, []).append((v, qk, i))

    per_queue = {}
    for ins in seq:
        qk, i = qidx[id(ins)]
        per_queue.setdefault(qk, []).append(ins)

    def merge(a, b):
        if not b:
            return a
        out = dict(a)
        for k, v in b.items():
            if out.get(k, 0) < v:
                out[k] = v
        return out

    comp_cache = {}

    def know_comp(qk, i):
        if i < 0:
            return {}
        key = (qk, i)
        if key in comp_cache:
            return comp_cache[key]
        know = dict(know_comp(qk, i - 1))
        ins = per_queue[qk][i]
        si = getattr(ins, "sync_info", None)
        if si:
            for w in (si.on_wait or []):
                if know.get(w.ant_name, 0) < w.wait_value:
                    know[w.ant_name] = w.wait_value
                    know = merge(know, know_from_obs(w.ant_name, w.wait_value))
        comp_cache[key] = know
        return know

    obs_cache = {}

    def _dma_sem(sem):
        return "DMA" in sem

    def know_from_obs(sem, v):
        if _dma_sem(sem):
            return {}
        key = (sem, v)
        if key in obs_cache:
            return obs_cache[key]
        obs_cache[key] = {}
        pts = incpoints.get(sem, [])
        know = {}
        if pts and all(q == pts[0][1] for _, q, _ in pts):
            for cnt, qk, i in pts:
                if cnt >= v:
                    if qk.startswith("E"):
                        know = dict(know_comp(qk, i))
                    know[sem] = cnt
                    break
        obs_cache[key] = know
        return know

    import os
    mode = os.environ.get("KERNEL_ELIM", "self")
    self_only = (mode == "self")

    def _same_queue_sem(sem, qk):
        pts = incpoints.get(sem, [])
        return bool(pts) and all(q == qk for _, q, _ in pts)

    dropped = 0
    kept = 0
    for qk, insts in per_queue.items():
        if not qk.startswith("E"):
            continue
        know = {}
        for ins in insts:
            si = getattr(ins, "sync_info", None)
            if not si:
                continue
            if type(ins).__name__ in ("InstDMACopy", "InstTensorLoad",
                                      "InstTensorSave", "InstTriggeredCopy"):
                continue
            waits = list(si.on_wait or [])
            if waits:
                changed = True
                waitset = waits[:]
                while changed:
                    changed = False
                    for w in waitset[:]:
                        if self_only and not _same_queue_sem(w.ant_name, qk):
                            continue
                        base = dict(know)
                        for w2 in waitset:
                            if w2 is w:
                                continue
                            base[w2.ant_name] = max(
                                base.get(w2.ant_name, 0), w2.wait_value)
                            base = merge(
                                base, know_from_obs(w2.ant_name, w2.wait_value))
                        if base.get(w.ant_name, 0) >= w.wait_value:
                            waitset.remove(w)
                            dropped += 1
                            changed = True
                            break
                for w in waitset:
                    kept += 1
                    know[w.ant_name] = max(know.get(w.ant_name, 0), w.wait_value)
                    know = merge(know, know_from_obs(w.ant_name, w.wait_value))
                if len(waitset) != len(waits):
                    ins.sync_info = mybir.SyncInfo(
                        on_wait=waitset, on_update=list(si.on_update or []))
    return dropped, kept


_SPLIT_WAIT_TYPES = {
    "InstMatmult", "InstTensorTensor", "InstTensorCopy", "InstActivation",
    "InstTensorScalarPtr", "InstTensorScalar", "InstReciprocal",
    "InstTensorReduce", "InstMemSet", "InstLdweights", "InstTranspose",
    "InstTensorTensorScan", "InstSelect", "InstCopy", "InstDMACopy",
    "InstTensorLoad", "InstTensorSave", "InstDrain",
}


def _split_matmul_waits(nc):
    """Walrus's TRN2 codegen allows at most one sync-wait per compute
    instruction.  Hoist every wait of a multi-wait instruction onto NoOps
    placed right before it on the same engine — the NX sequencer executes
    them in order, so semantics are identical.
    """
    cnt = 0
    for blk in nc.m.functions[0].blocks:
        insts = blk.instructions
        new = []
        for ins in insts:
            si = getattr(ins, "sync_info", None)
            if (type(ins).__name__ in _SPLIT_WAIT_TYPES and si is not None
                    and si.on_wait and len(si.on_wait) > 1):
                for j, w in enumerate(si.on_wait):
                    nop = mybir.InstNoOp(
                        name=f"{ins.name}-w{j}",
                        engine=ins.engine,
                        sync_info=mybir.SyncInfo(on_wait=[w], on_update=[]),
                        bass_nofuse=True,
                    )
                    new.append(nop)
                ins.sync_info = mybir.SyncInfo(
                    on_wait=[], on_update=list(si.on_update))
                cnt += 1
            new.append(ins)
        blk.instructions = new
    return cnt


def build_nc_full():
    nc = bass.Bass()
    # Per-core inputs.  x rows: pair p occupies partitions [0:128) as
    # (batch 2p on 0-63, batch 2p+1 on 64-127) after slicing.
    x32 = nc.declare_dram_parameter("x32", [BPC * C, N], F32, isOutput=False)
    x16 = nc.declare_dram_parameter("x16", [BPC * C, N], BF, isOutput=False)
    # four [128,128] block-diagonal conv weights packed along the free axis:
    # [bd(w_theta^T) | bd(w_phi^T) | bd(w_gv^T) | bd(w_mask^T)]
    wbdT = nc.declare_dram_parameter("wbdT", [128, 4 * 128], BF,
                                     isOutput=False)
    # w_mk^T in k-quarter-major layout [4*N, 256]
    wmkhT = nc.declare_dram_parameter("wmkhT", [4 * N, 256], BF,
                                      isOutput=False)
    out_e = nc.declare_dram_parameter("out", [BPC * C, N], F32, isOutput=True)

    with tile.TileContext(nc) as tc:
        with (
            tc.tile_pool(name="consts", bufs=1) as consts,
            tc.tile_pool(name="acts", bufs=2) as acts,
            tc.tile_pool(name="bigacts", bufs=2) as bigacts,
            tc.tile_pool(name="psO", bufs=2, space="PSUM") as psO_pool,
            tc.tile_pool(name="psS", bufs=2, space="PSUM") as psS,
            tc.tile_pool(name="psSm", bufs=2, space="PSUM") as psSm,
        ):
            _build_body(nc, tc, consts, acts, bigacts, psO_pool, psS, psSm,
                        x32, x16, wbdT, wmkhT, out_e)
    import os
    if os.environ.get("KERNEL_ELIM", "1") != "0":
        d, k = _eliminate_redundant_waits(nc)
        print(f"wait elimination: dropped {d}, kept {k}")
    _split_matmul_waits(nc)
    return nc


def _prep_weights(w_phi, w_theta, w_g, w_mask, w_mv, w_mk):
    bf = ml_dtypes.bfloat16
    z = np.zeros((C, C), np.float32)

    def bd(a):  # [64, 64] -> [128, 128] block-diagonal of a.T
        at = np.ascontiguousarray(a.T).astype(np.float32)
        return np.block([[at, z], [z, at]])

    w_gv = (w_mv.astype(np.float64) @ w_g.astype(np.float64)).astype(np.float32)
    wbd = np.concatenate(
        [bd(w_theta), bd(w_phi), bd(w_gv), bd(w_mask)], axis=1).astype(bf)
    # w_mk^T [m, k] -> k-quarter-major [4, m, 256] -> [4*m, 256]
    wmkT = np.ascontiguousarray(w_mk.T).astype(bf)
    wmkh = np.ascontiguousarray(
        wmkT.reshape(N, 4, 256).transpose(1, 0, 2)).reshape(4 * N, 256)
    return {
        "wbdT": np.ascontiguousarray(wbd),
        "wmkhT": wmkh,
    }


def kernel(x, w_phi, w_theta, w_g, w_mask, w_mv, w_mk, _trace=False):
    bf = ml_dtypes.bfloat16
    x = np.asarray(x, dtype=np.float32)
    weights = _prep_weights(np.asarray(w_phi, np.float32),
                            np.asarray(w_theta, np.float32),
                            np.asarray(w_g, np.float32),
                            np.asarray(w_mask, np.float32),
                            np.asarray(w_mv, np.float32),
                            np.asarray(w_mk, np.float32))

    xr = x.reshape(B, C, N)
    in_maps = []
    for i in range(NCORES):
        shard = np.ascontiguousarray(xr[i * BPC:(i + 1) * BPC]).reshape(BPC * C, N)
        m = {"x32": shard, "x16": shard.astype(bf)}
        m.update(weights)
        in_maps.append(m)

    nc = build_nc_full()
    res = run_bass_kernel_spmd(nc, in_maps, list(range(NCORES)), trace=_trace)
    outs = [np.asarray(res.results[i]["out"]).reshape(BPC, C, HH, WW)
            for i in range(NCORES)]
    full = np.concatenate(outs, axis=0)
    if _trace:
        return full, res
    return full
